# revision 2
# baseline (speedup 1.0000x reference)
"""DCN-v1 (dense_mlp) Trainium2 kernel.

Measured: ~0.57-0.62 ms HW exec (8 cores), rel err (l2) ~1.15e-3 — ~3.4x
faster than the indirect-DMA version (2.04 ms), whose per-128-row
instruction cost was the floor.

Strategy (8 NeuronCores, SPMD, data-parallel over batch; 2048 rows/core):
  - Embedding lookups via InstDMAGatherAnt (mlp Q7 library): one instruction
    moves thousands of table rows (vs 128/instruction for indirect DMA whose
    ~1.45us fixed cost bound the previous kernel at ~2.0ms).
    single_packet=False — coalescing >64 descriptors into one SDMA packet
    wedges the exec unit (NRT_EXEC_UNIT_UNRECOVERABLE).
  - 4 SWDGE queues (queue_num spread matching Tile's 8 round-robin DMASW
    lanes): SDMA engines round-robin the 4 rings, overlapping the ~125ns
    per-descriptor read round-trips that serialize one queue
    (measured 7.9 -> 2.6 ns/lookup).
  - Tables stored fp32 as 2-row 256B "super-rows" (the dma_gather element
    minimum) with SIGNED int16 super-indices: sidx = (idx>>1) - 25000 in
    [-25000, 25000); the Q7 address math (IVP_MULUSAN) is signed, so the
    src AP is based at row 25000. Each gather list is padded with 16
    trailing sidx=0 entries so the ucode's trailing-negative trim can
    never drop real lookups.
  - Sub-row select (idx&1) on DVE: one broadcast-flag multiply + one
    contiguous fold add; multi-hot histories then sum via a pairwise add
    tree (contiguous 32-elem inner dim — strided tensor_reduce ran ~7x
    below the contiguous DVE rate).
  - x0 assembly + PSUM eviction on the Scalar engine (ACT); feature-major
    x0^T via PE transposes; collapsed CrossNet (one 448->5 projection +
    scalar recurrence); bf16 MLP with fp32 PSUM accumulation.
"""

import os
import sys

import numpy as np
import ml_dtypes

for _p in ("/opt/trn_rl_repo", os.path.expanduser("~/.axon_site/_ro/trn_rl_repo")):
    if os.path.isdir(_p) and _p not in sys.path:
        sys.path.append(_p)

B = 16384
N_CORES = 8
BL = B // N_CORES  # 2048 rows per core
DENSE = 64
N_OH, N_MH, HIST = 8, 4, 20
VOCAB = 100000
SROWS = VOCAB // 2  # 50000 2-row super-rows per field
SHALF = SROWS // 2  # 25000: signed-index base row
EMB = 32
IN_DIM = 448
HID = [1024, 512, 256]
CHUNK = 128  # samples per transpose chunk
NBLK = 512  # samples per matmul n-block
SBLK = 256  # samples per mh gather block
N_GB = BL // SBLK  # 8 gather blocks per core
MH_N = SBLK * HIST  # 5120 real idxs per mh gather
MH_NP = MH_N + 16  # padded with trailing sidx=0
OH_N = BL  # 2048 real idxs per oh gather
OH_NP = OH_N + 16
KS = [128, 128, 128, 64]  # k-tile sizes over the 448-dim input features
BF16 = ml_dtypes.bfloat16


def _wrap_idx(flat):
    """[n] int16 -> [128, n//16] wrapped: position t at [t%16, t//16],
    replicated across the 8 16-partition groups."""
    n = flat.shape[0]
    w = flat.reshape(n // 16, 16).T  # [16, n//16]
    return np.tile(w, (8, 1)).astype(np.int16)


def _build_program(c_consts, sig_bias):
    from contextlib import ExitStack

    import concourse.bass as bass
    import concourse.tile as tile
    from concourse import bacc, mybir
    from concourse.masks import make_identity

    dt = mybir.dt
    AF = mybir.ActivationFunctionType
    n_chunks_per_nb = NBLK // CHUNK  # 4
    n_nb = BL // NBLK  # 4

    nc = bacc.Bacc(dynamic_dma_scratch_size=65536, num_swdge_queues=4)
    # per field: [50000 super-rows | copy of the first 25000] — the copy makes
    # numpy-style negative indexing in CoreSim land on the same rows the HW's
    # signed offset from base row 25000 reaches.
    tab_d = nc.dram_tensor(
        "tab2", [N_OH + N_MH, SROWS + SHALF, 64], dt.float32, kind="ExternalInput"
    )
    dense_d = nc.dram_tensor(
        "dense", [128, BL // 128, DENSE], dt.bfloat16, kind="ExternalInput"
    )
    ohi_d = nc.dram_tensor(
        "ohidx", [128, N_OH, OH_NP // 16], dt.int16, kind="ExternalInput"
    )
    mhi_d = nc.dram_tensor(
        "mhidx", [128, N_GB, N_MH, MH_NP // 16], dt.int16, kind="ExternalInput"
    )
    ohf_d = nc.dram_tensor(
        "ohflg", [128, BL // 128, N_OH, 2], dt.float32, kind="ExternalInput"
    )
    mhf_d = nc.dram_tensor(
        "mhflg", [128, N_GB, N_MH, MH_N // 128, 2], dt.float32, kind="ExternalInput"
    )
    w1_d = nc.dram_tensor("w1p", [128, 4, 1024], dt.bfloat16, kind="ExternalInput")
    w2_d = nc.dram_tensor("w2p", [128, 8, 512], dt.bfloat16, kind="ExternalInput")
    w3_d = nc.dram_tensor("w3p", [128, 4, 256], dt.bfloat16, kind="ExternalInput")
    wsm_d = nc.dram_tensor("wsm", [128, 22], dt.bfloat16, kind="ExternalInput")
    bias_d = nc.dram_tensor("biasp", [128, 14], dt.float32, kind="ExternalInput")
    out_d = nc.dram_tensor("out", [128, BL // 128], dt.float32, kind="ExternalOutput")

    with ExitStack() as ctx:
        tc = ctx.enter_context(tile.TileContext(nc))
        wp = ctx.enter_context(tc.tile_pool(name="weights", bufs=1))
        ohrp = ctx.enter_context(tc.tile_pool(name="ohr", bufs=2))
        mhrp = ctx.enter_context(tc.tile_pool(name="mhr", bufs=4))
        prodp = ctx.enter_context(tc.tile_pool(name="prod", bufs=2))
        r1p = ctx.enter_context(tc.tile_pool(name="r1", bufs=2))
        poolp = ctx.enter_context(tc.tile_pool(name="pool", bufs=2))
        x0p = ctx.enter_context(tc.tile_pool(name="x0", bufs=2))
        xtp = ctx.enter_context(tc.tile_pool(name="xt", bufs=2))
        hp = ctx.enter_context(tc.tile_pool(name="h", bufs=1))
        recp = ctx.enter_context(tc.tile_pool(name="rec", bufs=2))
        ps_mm = ctx.enter_context(tc.tile_pool(name="psmm", bufs=3, space="PSUM"))
        ps_tr = ctx.enter_context(tc.tile_pool(name="pstr", bufs=2, space="PSUM"))
        ps_sm = ctx.enter_context(tc.tile_pool(name="pssm", bufs=1, space="PSUM"))
        ps_q2 = ctx.enter_context(tc.tile_pool(name="psq2", bufs=2, space="PSUM"))

        # --- resident weights / indices / flags ---
        w1_sb = wp.tile([128, 4, 1024], dt.bfloat16)
        nc.sync.dma_start(w1_sb[:], w1_d[:])
        w2_sb = wp.tile([128, 8, 512], dt.bfloat16)
        nc.sync.dma_start(w2_sb[:], w2_d[:])
        w3_sb = wp.tile([128, 4, 256], dt.bfloat16)
        nc.sync.dma_start(w3_sb[:], w3_d[:])
        wsm_sb = wp.tile([128, 22], dt.bfloat16)
        nc.sync.dma_start(wsm_sb[:], wsm_d[:])
        bias_sb = wp.tile([128, 14], dt.float32)
        nc.sync.dma_start(bias_sb[:], bias_d[:])
        ident = wp.tile([128, 128], dt.bfloat16)
        make_identity(nc, ident[:])
        dense_sb = wp.tile([128, BL // 128, DENSE], dt.bfloat16)
        nc.sync.dma_start(dense_sb[:], dense_d[:])
        ohi_sb = wp.tile([128, N_OH, OH_NP // 16], dt.int16)
        nc.sync.dma_start(ohi_sb[:], ohi_d[:])
        ohf_sb = wp.tile([128, BL // 128, N_OH, 2], dt.float32)
        nc.sync.dma_start(ohf_sb[:], ohf_d[:])
        out_sb = wp.tile([128, BL // 128], dt.float32)
        # one-hot selected rows, chunk-major: [p, chunk, field, 32] bf16
        ohsel = wp.tile([128, BL // 128, N_OH, EMB], dt.bfloat16)

        src = tab_d[:, SHALF : SHALF + SROWS, :]

        # ---- one-hot: gather whole-core per field, select sub-row ----
        for f in range(N_OH):
            ohraw = ohrp.tile([128, OH_NP // 128 + 1, 64], dt.float32, tag="ohraw")
            nc.gpsimd.dma_gather(
                out_ap=ohraw[:],
                in_ap=src[f, :, :],
                idxs_ap=ohi_sb[:, f, :],
                num_idxs=OH_NP,
                num_idxs_reg=OH_NP,
                elem_size=64,
                elem_step=64,
                transpose=False,
                single_packet=False,
                queue_num=f % 4,
            )
            ohprod = ohrp.tile([128, BL // 128, 64], dt.float32, tag="ohprod")
            nc.vector.tensor_mul(
                ohprod[:].rearrange("p c (s e) -> p c s e", s=2),
                ohraw[:, 0 : BL // 128, :].rearrange("p c (s e) -> p c s e", s=2),
                ohf_sb[:, :, f, :].broadcast_to([128, BL // 128, 2, EMB]),
            )
            nc.vector.tensor_add(
                ohsel[:, :, f, :], ohprod[:, :, 0:32], ohprod[:, :, 32:64]
            )

        # ---- multi-hot: per gather-block of 256 samples, per field ----
        # list position t = sb*2560 + h*128 + p -> (sample sb*128+p, hist h);
        # gather layout puts it at [p, sb*20+h], so each sample's 20 rows are
        # contiguous in one partition's free dim.
        pooled = {}
        for b in range(N_GB):
            pb = poolp.tile([128, 2, N_MH, EMB], dt.float32, tag="pooled")
            pooled[b] = pb
            mhf_sb = r1p.tile([128, N_MH, MH_N // 128, 2], dt.float32, tag="mhf")
            nc.sync.dma_start(mhf_sb[:], mhf_d[:, b])
            mhi_sb = r1p.tile([128, N_MH, MH_NP // 16], dt.int16, tag="mhi")
            nc.sync.dma_start(mhi_sb[:], mhi_d[:, b])
            for f in range(N_MH):
                mhraw = mhrp.tile(
                    [128, MH_NP // 128 + 1, 64], dt.float32, tag="mhraw"
                )
                nc.gpsimd.dma_gather(
                    out_ap=mhraw[:],
                    in_ap=src[N_OH + f, :, :],
                    idxs_ap=mhi_sb[:, f, :],
                    num_idxs=MH_NP,
                    num_idxs_reg=MH_NP,
                    elem_size=64,
                    elem_step=64,
                    transpose=False,
                    single_packet=False,
                    queue_num=(b * N_MH + f) % 4,
                )
                prod = prodp.tile([128, MH_N // 128, 64], dt.float32, tag="mhprod")
                nc.vector.tensor_mul(
                    prod[:].rearrange("p j (s e) -> p j s e", s=2),
                    mhraw[:, 0 : MH_N // 128, :].rearrange(
                        "p j (s e) -> p j s e", s=2
                    ),
                    mhf_sb[:, f, :, :].broadcast_to([128, MH_N // 128, 2, EMB]),
                )
                # fold the 2 sub-rows, then a pairwise tree over hist — every
                # add has a contiguous 32-elem inner dim.
                s2 = r1p.tile([128, MH_N // 128, 32], dt.bfloat16, tag="s2")
                nc.vector.tensor_add(s2[:], prod[:, :, 0:32], prod[:, :, 32:64])
                s2v = s2[:].rearrange("p (sb h) e -> p sb h e", sb=2)
                t1 = r1p.tile([128, 2, 10, 32], dt.bfloat16, tag="t1")
                nc.vector.tensor_add(t1[:], s2v[:, :, 0::2, :], s2v[:, :, 1::2, :])
                t2 = r1p.tile([128, 2, 5, 32], dt.bfloat16, tag="t2")
                nc.vector.tensor_add(t2[:], t1[:, :, 0::2, :], t1[:, :, 1::2, :])
                t3 = r1p.tile([128, 2, 2, 32], dt.bfloat16, tag="t3")
                nc.vector.tensor_add(t3[:], t2[:, :, 0:4:2, :], t2[:, :, 1:4:2, :])
                t4 = r1p.tile([128, 2, 32], dt.float32, tag="t4")
                nc.vector.tensor_add(t4[:], t3[:, :, 0, :], t3[:, :, 1, :])
                nc.vector.tensor_add(pb[:, :, f, :], t4[:], t2[:, :, 4, :])

        # ---- per n-block: build x0^T, cross projections, MLP, output ----
        for nb in range(n_nb):
            x0T = xtp.tile([128, 4, NBLK], dt.bfloat16, tag="x0T")
            lgq1 = recp.tile([128, 4], dt.float32, tag="lgq1")
            for cc in range(n_chunks_per_nb):
                c = nb * n_chunks_per_nb + cc
                cs = slice(cc * CHUNK, (cc + 1) * CHUNK)
                b, sb = c // 2, c % 2

                x0c = x0p.tile([128, 512], dt.bfloat16, tag="x0c")
                nc.vector.memset(x0c[:, 448:512], 0.0)
                nc.scalar.activation(
                    x0c[:, 0:DENSE], dense_sb[:, c, :], AF.Copy, bias=0.0
                )
                nc.scalar.activation(
                    x0c[:, DENSE : DENSE + N_OH * EMB].rearrange(
                        "p (f e) -> p f e", f=N_OH
                    ),
                    ohsel[:, c, :, :],
                    AF.Copy,
                    bias=0.0,
                )
                nc.scalar.activation(
                    x0c[:, 320:448].rearrange("p (f e) -> p f e", f=N_MH),
                    pooled[b][:, sb, :, :],
                    AF.Copy,
                    bias=0.0,
                )

                tp = ps_tr.tile([128, 4, 128], dt.bfloat16, tag="trps")
                for j in range(4):
                    nc.tensor.transpose(
                        tp[:, j : j + 1, :],
                        x0c[:, j * 128 : (j + 1) * 128],
                        ident[:],
                    )
                nc.scalar.activation(x0T[:, :, cs], tp[:], AF.Copy, bias=0.0)

                pn = ps_sm.tile([128, 5], dt.float32, tag="pn")
                for j in range(4):
                    nc.tensor.matmul(
                        pn[:],
                        x0T[0 : KS[j], j : j + 1, cs],
                        wsm_sb[0 : KS[j], j * 5 : j * 5 + 5],
                        start=(j == 0),
                        stop=(j == 3),
                    )
                pp1 = recp.tile([128, 4], dt.float32, tag="pp1")
                nc.vector.tensor_scalar_add(pp1[:], pn[:, 0:4], 1.0)
                m01 = recp.tile([128, 1], dt.float32, tag="m01")
                nc.vector.tensor_mul(m01[:], pp1[:, 0:1], pp1[:, 1:2])
                m23 = recp.tile([128, 1], dt.float32, tag="m23")
                nc.vector.tensor_mul(m23[:], pp1[:, 2:3], pp1[:, 3:4])
                a4 = recp.tile([128, 1], dt.float32, tag="a4")
                nc.vector.tensor_mul(a4[:], m01[:], m23[:])
                nc.vector.tensor_mul(lgq1[:, cc : cc + 1], a4[:], pn[:, 4:5])

            # ---- deep net ----
            h1 = hp.tile([128, 8, NBLK], dt.bfloat16, tag="h1")
            for m in range(8):
                ps = ps_mm.tile([128, NBLK], dt.float32, tag="mm")
                for j in range(4):
                    nc.tensor.matmul(
                        ps[:],
                        w1_sb[0 : KS[j], j : j + 1, m * 128 : (m + 1) * 128],
                        x0T[0 : KS[j], j : j + 1, :],
                        start=(j == 0),
                        stop=(j == 3),
                    )
                nc.scalar.activation(
                    h1[:, m : m + 1, :], ps[:], AF.Relu, bias=bias_sb[:, m : m + 1]
                )
            h2 = hp.tile([128, 4, NBLK], dt.bfloat16, tag="h2")
            for m in range(4):
                ps = ps_mm.tile([128, NBLK], dt.float32, tag="mm")
                for j in range(8):
                    nc.tensor.matmul(
                        ps[:],
                        w2_sb[:, j : j + 1, m * 128 : (m + 1) * 128],
                        h1[:, j : j + 1, :],
                        start=(j == 0),
                        stop=(j == 7),
                    )
                nc.scalar.activation(
                    h2[:, m : m + 1, :], ps[:], AF.Relu, bias=bias_sb[:, 8 + m : 9 + m]
                )
            h3 = hp.tile([128, 2, NBLK], dt.bfloat16, tag="h3")
            for m in range(2):
                ps = ps_mm.tile([128, NBLK], dt.float32, tag="mm")
                for j in range(4):
                    nc.tensor.matmul(
                        ps[:],
                        w3_sb[:, j : j + 1, m * 128 : (m + 1) * 128],
                        h2[:, j : j + 1, :],
                        start=(j == 0),
                        stop=(j == 3),
                    )
                nc.scalar.activation(
                    h3[:, m : m + 1, :],
                    ps[:],
                    AF.Relu,
                    bias=bias_sb[:, 12 + m : 13 + m],
                )

            # ---- final: logit = prod(1+p)*q1 + h3.lin_w_bot + sig_bias ----
            for cc in range(n_chunks_per_nb):
                c = nb * n_chunks_per_nb + cc
                cs = slice(cc * CHUNK, (cc + 1) * CHUNK)
                q2n = ps_q2.tile([128, 1], dt.float32, tag="q2n")
                for j in range(2):
                    nc.tensor.matmul(
                        q2n[:],
                        h3[:, j : j + 1, cs],
                        wsm_sb[:, 20 + j : 21 + j],
                        start=(j == 0),
                        stop=(j == 1),
                    )
                lg2 = recp.tile([128, 1], dt.float32, tag="lg2")
                nc.vector.tensor_add(lg2[:], lgq1[:, cc : cc + 1], q2n[:])
                nc.scalar.activation(
                    out_sb[:, c : c + 1], lg2[:], AF.Sigmoid, bias=float(sig_bias)
                )

        nc.sync.dma_start(out_d[:], out_sb[:])

    nc.compile()
    # queue_num must match Tile's round-robin DMASW sem-lane assignment
    # (visit order != emission order), or one sem lane would see increments
    # from two SWDGE queues.
    for blk in nc.main_func.blocks:
        for inst in blk.instructions:
            if type(inst).__name__ == "InstDMAGatherAnt":
                si = inst.sync_info
                assert si is not None and si.on_update, inst.name
                name = si.on_update[0].ant_name
                assert name.startswith("DMASW"), name
                inst.queue_num = int(name[5:].split("_")[0]) % 4
    return nc


def _prep_inputs(
    dense_x,
    one_hot_x,
    multi_hot_x,
    one_hot_emb,
    multi_hot_emb,
    cross_w,
    cross_b,
    W1,
    b1,
    W2,
    b2,
    W3,
    b3,
    lin_w,
    lin_b,
):
    dense_bf = np.ascontiguousarray(dense_x, dtype=np.float32).astype(BF16)
    # fp32 2-row super-row tables with a wraparound copy of the low half
    tab = np.concatenate(
        [
            np.asarray(one_hot_emb, np.float32).reshape(N_OH, VOCAB, EMB),
            np.asarray(multi_hot_emb, np.float32).reshape(N_MH, VOCAB, EMB),
        ]
    ).reshape(N_OH + N_MH, SROWS, 64)
    tab2 = np.ascontiguousarray(
        np.concatenate([tab, tab[:, :SHALF, :]], axis=1), dtype=np.float32
    )

    oh_idx = np.asarray(one_hot_x, np.int64)  # (B, 8)
    mh_idx = np.asarray(multi_hot_x, np.int64)  # (B, 4, 20)

    def pack_k(Wmat, out_cols):
        p = np.zeros((128, 4, out_cols), np.float32)
        for j in range(4):
            p[0 : KS[j], j, :] = Wmat[j * 128 : j * 128 + KS[j], :]
        return p.astype(BF16)

    w1p = pack_k(np.asarray(W1, np.float32), 1024)
    w2p = (
        np.asarray(W2, np.float32)
        .reshape(8, 128, 512)
        .transpose(1, 0, 2)
        .copy()
        .astype(BF16)
    )
    w3p = (
        np.asarray(W3, np.float32)
        .reshape(4, 128, 256)
        .transpose(1, 0, 2)
        .copy()
        .astype(BF16)
    )
    lw = np.asarray(lin_w, np.float32)[:, 0]
    cwq = pack_k(
        np.concatenate([np.asarray(cross_w, np.float32).T, lw[:IN_DIM, None]], 1), 5
    )
    wsm = np.zeros((128, 22), np.float32)
    wsm[:, 0:20] = cwq.astype(np.float32).reshape(128, 20)
    wsm[:, 20:22] = lw[IN_DIM:].reshape(2, 128).T
    wsm = wsm.astype(BF16)
    biasp = np.concatenate(
        [
            np.asarray(b1, np.float32).reshape(8, 128).T,
            np.asarray(b2, np.float32).reshape(4, 128).T,
            np.asarray(b3, np.float32).reshape(2, 128).T,
        ],
        axis=1,
    ).copy()

    cb = np.asarray(cross_b, np.float64)
    cwf = np.asarray(cross_w, np.float64)
    C = np.zeros(IN_DIM, np.float64)
    c_consts = []
    for l in range(4):
        c_consts.append(float(C @ cwf[l]))
        C = C + cb[l]
    sig_bias = float(C @ np.asarray(lw[:IN_DIM], np.float64)) + float(
        np.asarray(lin_b, np.float64).reshape(-1)[0]
    )
    if any(abs(c) > 1e-30 for c in c_consts):
        raise NotImplementedError("cross_b != 0 unsupported (always 0 here)")

    shared = {
        "tab2": tab2,
        "w1p": w1p,
        "w2p": w2p,
        "w3p": w3p,
        "wsm": wsm,
        "biasp": biasp,
    }
    pad = np.zeros(16, np.int16)
    in_maps = []
    for core in range(N_CORES):
        rs = slice(core * BL, (core + 1) * BL)
        m = dict(shared)
        m["dense"] = np.ascontiguousarray(
            dense_bf[rs].reshape(BL // 128, 128, DENSE).transpose(1, 0, 2)
        )
        # one-hot: list position t = sample_local; out[p, c] = sample c*128+p
        soh = oh_idx[rs]  # (BL, 8)
        ohidx = np.stack(
            [
                _wrap_idx(
                    np.concatenate([((soh[:, f] >> 1) - SHALF).astype(np.int16), pad])
                )
                for f in range(N_OH)
            ],
            axis=1,
        )  # [128, 8, 129]
        m["ohidx"] = np.ascontiguousarray(ohidx)
        sub_oh = (soh & 1).reshape(BL // 128, 128, N_OH).transpose(1, 0, 2)
        m["ohflg"] = np.ascontiguousarray(
            (sub_oh[:, :, :, None] == np.arange(2)).astype(np.float32)
        )  # [128, 16, 8, 2]
        # multi-hot: per block b, field f: t = sb*2560 + h*128 + p
        smh = mh_idx[rs]  # (BL, 4, 20)
        mhidx = np.zeros((128, N_GB, N_MH, MH_NP // 16), np.int16)
        mhflg = np.zeros((128, N_GB, N_MH, MH_N // 128, 2), np.float32)
        for bb in range(N_GB):
            chunk = smh[bb * SBLK : (bb + 1) * SBLK]  # (256, 4, 20)
            for f in range(N_MH):
                l3 = chunk[:, f, :].reshape(2, 128, HIST).transpose(0, 2, 1)
                # l3[sb, h, p] = idx of (sample sb*128+p, hist h)
                flat = ((l3 >> 1) - SHALF).reshape(MH_N).astype(np.int16)
                mhidx[:, bb, f, :] = _wrap_idx(np.concatenate([flat, pad]))
                sub = (l3 & 1).transpose(2, 0, 1).reshape(128, MH_N // 128)
                mhflg[:, bb, f, :, :] = (sub[:, :, None] == np.arange(2)).astype(
                    np.float32
                )
        m["mhidx"] = np.ascontiguousarray(mhidx)
        m["mhflg"] = np.ascontiguousarray(mhflg)
        in_maps.append(m)
    return in_maps, c_consts, sig_bias


def _patch_interp_gather():
    """Dev-only CoreSim patch: the stock interp asserts gather indices are
    >= -1; our signed indices are valid on HW (signed Q7 address math) and
    the wraparound table copy makes numpy negative indexing agree."""
    import einops
    from concourse import bass_interp
    from concourse.bass_interp import Direction

    def cdiv(a, bb):
        return -(-a // bb)

    def patched(self, ins, captured, *, reg_snapshot):
        src_ap = self.view_ap(
            ins.ins[:-2], Direction.READ, ins, reg_snapshot=reg_snapshot
        )
        idxs_ap, _ = captured
        dst_ap = self.view_ap(
            ins.outs[0], Direction.WRITE, ins, reg_snapshot=reg_snapshot
        )
        assert not ins.transpose
        src = src_ap.reshape((-1, ins.elem_size))
        idxs = idxs_ap.reshape((128, cdiv(ins.num_idxs, 16)))
        dst = dst_ap.reshape((128, cdiv(ins.num_idxs, 128), ins.elem_size))
        unwrapped = einops.rearrange(idxs[:16, :], "p s -> (s p)")[: ins.num_idxs]
        gathered = src[unwrapped]
        n = ins.num_idxs
        full = n // 128 * 128
        if full:
            dst[:, : n // 128, :] = (
                gathered[:full]
                .reshape(n // 128, 128, ins.elem_size)
                .transpose(1, 0, 2)
            )
        for i in range(full, n):
            dst[i % 128, i // 128, :] = gathered[i]

    bass_interp.InstructionExecutor._exec_InstDMAGatherAnt = patched


def _run(inputs, trace=False, sim=False):
    from concourse.bass_utils import run_bass_kernel_spmd

    in_maps, c_consts, sig_bias = _prep_inputs(**inputs)
    nc = _build_program(c_consts, sig_bias)
    if sim:
        _patch_interp_gather()
        from concourse.bass_interp import CoreSim

        csim = CoreSim(nc, trace=False)
        for k, v in in_maps[0].items():
            csim.tensor(k)[:] = v
        csim.simulate()
        out0 = np.asarray(csim.tensor("out"))
        outs = [out0.reshape(128, BL // 128).T.reshape(BL)]
        full = np.concatenate(outs).reshape(-1, 1).astype(np.float32)
        return full, None
    res = run_bass_kernel_spmd(nc, in_maps, core_ids=list(range(N_CORES)), trace=trace)
    outs = [
        res.results[c]["out"].reshape(128, BL // 128).T.reshape(BL)
        for c in range(N_CORES)
    ]
    full = np.concatenate(outs).reshape(B, 1).astype(np.float32)
    return full, res


def kernel(**inputs):
    full, _ = _run(inputs, trace=False)
    return full


# revision 4
# speedup vs baseline: 1.1206x; 1.1206x over previous
"""DCN-v1 (dense_mlp) Trainium2 kernel.

Measured: ~0.57-0.62 ms HW exec (8 cores), rel err (l2) ~1.15e-3 — ~3.4x
faster than the indirect-DMA version (2.04 ms), whose per-128-row
instruction cost was the floor.

Strategy (8 NeuronCores, SPMD, data-parallel over batch; 2048 rows/core):
  - Embedding lookups via InstDMAGatherAnt (mlp Q7 library): one instruction
    moves thousands of table rows (vs 128/instruction for indirect DMA whose
    ~1.45us fixed cost bound the previous kernel at ~2.0ms).
    single_packet=False — coalescing >64 descriptors into one SDMA packet
    wedges the exec unit (NRT_EXEC_UNIT_UNRECOVERABLE).
  - 4 SWDGE queues (queue_num spread matching Tile's 8 round-robin DMASW
    lanes): SDMA engines round-robin the 4 rings, overlapping the ~125ns
    per-descriptor read round-trips that serialize one queue
    (measured 7.9 -> 2.6 ns/lookup).
  - Tables stored fp32 as 2-row 256B "super-rows" (the dma_gather element
    minimum) with SIGNED int16 super-indices: sidx = (idx>>1) - 25000 in
    [-25000, 25000); the Q7 address math (IVP_MULUSAN) is signed, so the
    src AP is based at row 25000. Each gather list is padded with 16
    trailing sidx=0 entries so the ucode's trailing-negative trim can
    never drop real lookups.
  - Sub-row select (idx&1) on DVE: one broadcast-flag multiply + one
    contiguous fold add; multi-hot histories then sum via a pairwise add
    tree (contiguous 32-elem inner dim — strided tensor_reduce ran ~7x
    below the contiguous DVE rate).
  - x0 assembly + PSUM eviction on the Scalar engine (ACT); feature-major
    x0^T via PE transposes; collapsed CrossNet (one 448->5 projection +
    scalar recurrence); bf16 MLP with fp32 PSUM accumulation.
"""

import os
import sys

import numpy as np
import ml_dtypes

for _p in ("/opt/trn_rl_repo", os.path.expanduser("~/.axon_site/_ro/trn_rl_repo")):
    if os.path.isdir(_p) and _p not in sys.path:
        sys.path.append(_p)

B = 16384
N_CORES = 8
BL = B // N_CORES  # 2048 rows per core
DENSE = 64
N_OH, N_MH, HIST = 8, 4, 20
VOCAB = 100000
SROWS = VOCAB // 2  # 50000 2-row super-rows per field
SHALF = SROWS // 2  # 25000: signed-index base row
EMB = 32
IN_DIM = 448
HID = [1024, 512, 256]
CHUNK = 128  # samples per transpose chunk
NBLK = 512  # samples per matmul n-block
SBLK = 256  # samples per mh gather block
N_GB = BL // SBLK  # 8 gather blocks per core
MH_N = SBLK * HIST  # 5120 real idxs per mh gather
MH_NP = MH_N + 16  # padded with trailing sidx=0
OH_N = BL  # 2048 real idxs per oh gather
OH_NP = OH_N + 16
KS = [128, 128, 128, 64]  # k-tile sizes over the 448-dim input features
BF16 = ml_dtypes.bfloat16


def _wrap_idx(flat):
    """[n] int16 -> [128, n//16] wrapped: position t at [t%16, t//16],
    replicated across the 8 16-partition groups."""
    n = flat.shape[0]
    w = flat.reshape(n // 16, 16).T  # [16, n//16]
    return np.tile(w, (8, 1)).astype(np.int16)


def _build_program(c_consts, sig_bias):
    from contextlib import ExitStack

    import concourse.bass as bass
    import concourse.tile as tile
    from concourse import bacc, mybir
    from concourse.masks import make_identity

    dt = mybir.dt
    AF = mybir.ActivationFunctionType
    n_chunks_per_nb = NBLK // CHUNK  # 4
    n_nb = BL // NBLK  # 4

    nc = bacc.Bacc(dynamic_dma_scratch_size=49152, num_swdge_queues=4)
    # per field: [50000 super-rows | copy of the first 25000] — the copy makes
    # numpy-style negative indexing in CoreSim land on the same rows the HW's
    # signed offset from base row 25000 reaches.
    tab_d = nc.dram_tensor(
        "tab2", [N_OH + N_MH, SROWS + SHALF, 64], dt.float32, kind="ExternalInput"
    )
    dense_d = nc.dram_tensor(
        "dense", [128, BL // 128, DENSE], dt.bfloat16, kind="ExternalInput"
    )
    ohi_d = nc.dram_tensor(
        "ohidx", [128, N_OH, OH_NP // 16], dt.int16, kind="ExternalInput"
    )
    mhi_d = nc.dram_tensor(
        "mhidx", [128, N_GB, N_MH, MH_NP // 16], dt.int16, kind="ExternalInput"
    )
    ohf_d = nc.dram_tensor(
        "ohflg", [128, BL // 128, N_OH, 2], dt.float32, kind="ExternalInput"
    )
    mhf_d = nc.dram_tensor(
        "mhflg", [128, N_GB, N_MH, MH_N // 128, 2], dt.float32, kind="ExternalInput"
    )
    w1_d = nc.dram_tensor("w1p", [128, 4, 1024], dt.bfloat16, kind="ExternalInput")
    w2_d = nc.dram_tensor("w2p", [128, 8, 512], dt.bfloat16, kind="ExternalInput")
    w3_d = nc.dram_tensor("w3p", [128, 4, 256], dt.bfloat16, kind="ExternalInput")
    wsm_d = nc.dram_tensor("wsm", [128, 22], dt.bfloat16, kind="ExternalInput")
    bias_d = nc.dram_tensor("biasp", [128, 14], dt.float32, kind="ExternalInput")
    out_d = nc.dram_tensor("out", [128, BL // 128], dt.float32, kind="ExternalOutput")

    with ExitStack() as ctx:
        tc = ctx.enter_context(tile.TileContext(nc))
        wp = ctx.enter_context(tc.tile_pool(name="weights", bufs=1))
        ohrp = ctx.enter_context(tc.tile_pool(name="ohr", bufs=2))
        mhrp = ctx.enter_context(tc.tile_pool(name="mhr", bufs=5))
        prodp = ctx.enter_context(tc.tile_pool(name="prod", bufs=3))
        r1p = ctx.enter_context(tc.tile_pool(name="r1", bufs=2))
        poolp = ctx.enter_context(tc.tile_pool(name="pool", bufs=2))
        x0p = ctx.enter_context(tc.tile_pool(name="x0", bufs=2))
        xtp = ctx.enter_context(tc.tile_pool(name="xt", bufs=2))
        hp = ctx.enter_context(tc.tile_pool(name="h", bufs=1))
        recp = ctx.enter_context(tc.tile_pool(name="rec", bufs=2))
        ps_mm = ctx.enter_context(tc.tile_pool(name="psmm", bufs=3, space="PSUM"))
        ps_tr = ctx.enter_context(tc.tile_pool(name="pstr", bufs=2, space="PSUM"))
        ps_sm = ctx.enter_context(tc.tile_pool(name="pssm", bufs=1, space="PSUM"))
        ps_q2 = ctx.enter_context(tc.tile_pool(name="psq2", bufs=2, space="PSUM"))

        # --- resident weights / indices / flags ---
        w1_sb = wp.tile([128, 4, 1024], dt.bfloat16)
        nc.sync.dma_start(w1_sb[:], w1_d[:])
        w2_sb = wp.tile([128, 8, 512], dt.bfloat16)
        nc.sync.dma_start(w2_sb[:], w2_d[:])
        w3_sb = wp.tile([128, 4, 256], dt.bfloat16)
        nc.sync.dma_start(w3_sb[:], w3_d[:])
        wsm_sb = wp.tile([128, 22], dt.bfloat16)
        nc.sync.dma_start(wsm_sb[:], wsm_d[:])
        bias_sb = wp.tile([128, 14], dt.float32)
        nc.sync.dma_start(bias_sb[:], bias_d[:])
        ident = wp.tile([128, 128], dt.bfloat16)
        make_identity(nc, ident[:])
        dense_sb = wp.tile([128, BL // 128, DENSE], dt.bfloat16)
        nc.sync.dma_start(dense_sb[:], dense_d[:])
        ohi_sb = wp.tile([128, N_OH, OH_NP // 16], dt.int16)
        nc.sync.dma_start(ohi_sb[:], ohi_d[:])
        ohf_sb = wp.tile([128, BL // 128, N_OH, 2], dt.float32)
        nc.sync.dma_start(ohf_sb[:], ohf_d[:])
        out_sb = wp.tile([128, BL // 128], dt.float32)
        # one-hot selected rows, chunk-major: [p, chunk, field, 32] bf16
        ohsel = wp.tile([128, BL // 128, N_OH, EMB], dt.bfloat16)

        src = tab_d[:, SHALF : SHALF + SROWS, :]

        # ---- one-hot: gather whole-core per field, select sub-row ----
        for f in range(N_OH):
            ohraw = ohrp.tile([128, OH_NP // 128 + 1, 64], dt.float32, tag="ohraw")
            nc.gpsimd.dma_gather(
                out_ap=ohraw[:],
                in_ap=src[f, :, :],
                idxs_ap=ohi_sb[:, f, :],
                num_idxs=OH_NP,
                num_idxs_reg=OH_NP,
                elem_size=64,
                elem_step=64,
                transpose=False,
                single_packet=False,
                queue_num=f % 4,
            )
            ohprod = ohrp.tile([128, BL // 128, 64], dt.float32, tag="ohprod")
            nc.vector.tensor_mul(
                ohprod[:].rearrange("p c (s e) -> p c s e", s=2),
                ohraw[:, 0 : BL // 128, :].rearrange("p c (s e) -> p c s e", s=2),
                ohf_sb[:, :, f, :].broadcast_to([128, BL // 128, 2, EMB]),
            )
            nc.vector.tensor_add(
                ohsel[:, :, f, :], ohprod[:, :, 0:32], ohprod[:, :, 32:64]
            )

        # ---- multi-hot: per gather-block of 256 samples, per field ----
        # list position t = sb*2560 + h*128 + p -> (sample sb*128+p, hist h);
        # gather layout puts it at [p, sb*20+h], so each sample's 20 rows are
        # contiguous in one partition's free dim.
        pooled = {}
        for b in range(N_GB):
            pb = poolp.tile([128, 2, N_MH, EMB], dt.float32, tag="pooled")
            pooled[b] = pb
            mhf_sb = r1p.tile([128, N_MH, MH_N // 128, 2], dt.float32, tag="mhf")
            nc.sync.dma_start(mhf_sb[:], mhf_d[:, b])
            mhi_sb = r1p.tile([128, N_MH, MH_NP // 16], dt.int16, tag="mhi")
            nc.sync.dma_start(mhi_sb[:], mhi_d[:, b])
            for f in range(N_MH):
                mhraw = mhrp.tile(
                    [128, MH_NP // 128 + 1, 64], dt.float32, tag="mhraw"
                )
                nc.gpsimd.dma_gather(
                    out_ap=mhraw[:],
                    in_ap=src[N_OH + f, :, :],
                    idxs_ap=mhi_sb[:, f, :],
                    num_idxs=MH_NP,
                    num_idxs_reg=MH_NP,
                    elem_size=64,
                    elem_step=64,
                    transpose=False,
                    single_packet=False,
                    queue_num=(b * N_MH + f) % 4,
                )
                prod = prodp.tile([128, MH_N // 128, 64], dt.float32, tag="mhprod")
                nc.vector.tensor_mul(
                    prod[:].rearrange("p j (s e) -> p j s e", s=2),
                    mhraw[:, 0 : MH_N // 128, :].rearrange(
                        "p j (s e) -> p j s e", s=2
                    ),
                    mhf_sb[:, f, :, :].broadcast_to([128, MH_N // 128, 2, EMB]),
                )
                # fold the 2 sub-rows, then a pairwise tree over hist — every
                # add has a contiguous 32-elem inner dim.
                s2 = r1p.tile([128, MH_N // 128, 32], dt.bfloat16, tag="s2")
                nc.vector.tensor_add(s2[:], prod[:, :, 0:32], prod[:, :, 32:64])
                s2v = s2[:].rearrange("p (sb h) e -> p sb h e", sb=2)
                t1 = r1p.tile([128, 2, 10, 32], dt.bfloat16, tag="t1")
                nc.vector.tensor_add(t1[:], s2v[:, :, 0::2, :], s2v[:, :, 1::2, :])
                t2 = r1p.tile([128, 2, 5, 32], dt.bfloat16, tag="t2")
                nc.vector.tensor_add(t2[:], t1[:, :, 0::2, :], t1[:, :, 1::2, :])
                t3 = r1p.tile([128, 2, 2, 32], dt.bfloat16, tag="t3")
                nc.vector.tensor_add(t3[:], t2[:, :, 0:4:2, :], t2[:, :, 1:4:2, :])
                t4 = r1p.tile([128, 2, 32], dt.float32, tag="t4")
                nc.vector.tensor_add(t4[:], t3[:, :, 0, :], t3[:, :, 1, :])
                nc.vector.tensor_add(pb[:, :, f, :], t4[:], t2[:, :, 4, :])

        # ---- per n-block: build x0^T, cross projections, MLP, output ----
        for nb in range(n_nb):
            x0T = xtp.tile([128, 4, NBLK], dt.bfloat16, tag="x0T")
            lgq1 = recp.tile([128, 4], dt.float32, tag="lgq1")
            for cc in range(n_chunks_per_nb):
                c = nb * n_chunks_per_nb + cc
                cs = slice(cc * CHUNK, (cc + 1) * CHUNK)
                b, sb = c // 2, c % 2

                x0c = x0p.tile([128, 512], dt.bfloat16, tag="x0c")
                nc.vector.memset(x0c[:, 448:512], 0.0)
                nc.scalar.activation(
                    x0c[:, 0:DENSE], dense_sb[:, c, :], AF.Copy, bias=0.0
                )
                nc.scalar.activation(
                    x0c[:, DENSE : DENSE + N_OH * EMB].rearrange(
                        "p (f e) -> p f e", f=N_OH
                    ),
                    ohsel[:, c, :, :],
                    AF.Copy,
                    bias=0.0,
                )
                nc.scalar.activation(
                    x0c[:, 320:448].rearrange("p (f e) -> p f e", f=N_MH),
                    pooled[b][:, sb, :, :],
                    AF.Copy,
                    bias=0.0,
                )

                tp = ps_tr.tile([128, 4, 128], dt.bfloat16, tag="trps")
                for j in range(4):
                    nc.tensor.transpose(
                        tp[:, j : j + 1, :],
                        x0c[:, j * 128 : (j + 1) * 128],
                        ident[:],
                    )
                nc.scalar.activation(x0T[:, :, cs], tp[:], AF.Copy, bias=0.0)

                pn = ps_sm.tile([128, 5], dt.float32, tag="pn")
                for j in range(4):
                    nc.tensor.matmul(
                        pn[:],
                        x0T[0 : KS[j], j : j + 1, cs],
                        wsm_sb[0 : KS[j], j * 5 : j * 5 + 5],
                        start=(j == 0),
                        stop=(j == 3),
                    )
                pp1 = recp.tile([128, 4], dt.float32, tag="pp1")
                nc.vector.tensor_scalar_add(pp1[:], pn[:, 0:4], 1.0)
                m01 = recp.tile([128, 1], dt.float32, tag="m01")
                nc.vector.tensor_mul(m01[:], pp1[:, 0:1], pp1[:, 1:2])
                m23 = recp.tile([128, 1], dt.float32, tag="m23")
                nc.vector.tensor_mul(m23[:], pp1[:, 2:3], pp1[:, 3:4])
                a4 = recp.tile([128, 1], dt.float32, tag="a4")
                nc.vector.tensor_mul(a4[:], m01[:], m23[:])
                nc.vector.tensor_mul(lgq1[:, cc : cc + 1], a4[:], pn[:, 4:5])

            # ---- deep net ----
            h1 = hp.tile([128, 8, NBLK], dt.bfloat16, tag="h1")
            for m in range(8):
                ps = ps_mm.tile([128, NBLK], dt.float32, tag="mm")
                for j in range(4):
                    nc.tensor.matmul(
                        ps[:],
                        w1_sb[0 : KS[j], j : j + 1, m * 128 : (m + 1) * 128],
                        x0T[0 : KS[j], j : j + 1, :],
                        start=(j == 0),
                        stop=(j == 3),
                    )
                nc.scalar.activation(
                    h1[:, m : m + 1, :], ps[:], AF.Relu, bias=bias_sb[:, m : m + 1]
                )
            h2 = hp.tile([128, 4, NBLK], dt.bfloat16, tag="h2")
            for m in range(4):
                ps = ps_mm.tile([128, NBLK], dt.float32, tag="mm")
                for j in range(8):
                    nc.tensor.matmul(
                        ps[:],
                        w2_sb[:, j : j + 1, m * 128 : (m + 1) * 128],
                        h1[:, j : j + 1, :],
                        start=(j == 0),
                        stop=(j == 7),
                    )
                nc.scalar.activation(
                    h2[:, m : m + 1, :], ps[:], AF.Relu, bias=bias_sb[:, 8 + m : 9 + m]
                )
            h3 = hp.tile([128, 2, NBLK], dt.bfloat16, tag="h3")
            for m in range(2):
                ps = ps_mm.tile([128, NBLK], dt.float32, tag="mm")
                for j in range(4):
                    nc.tensor.matmul(
                        ps[:],
                        w3_sb[:, j : j + 1, m * 128 : (m + 1) * 128],
                        h2[:, j : j + 1, :],
                        start=(j == 0),
                        stop=(j == 3),
                    )
                nc.scalar.activation(
                    h3[:, m : m + 1, :],
                    ps[:],
                    AF.Relu,
                    bias=bias_sb[:, 12 + m : 13 + m],
                )

            # ---- final: logit = prod(1+p)*q1 + h3.lin_w_bot + sig_bias ----
            for cc in range(n_chunks_per_nb):
                c = nb * n_chunks_per_nb + cc
                cs = slice(cc * CHUNK, (cc + 1) * CHUNK)
                q2n = ps_q2.tile([128, 1], dt.float32, tag="q2n")
                for j in range(2):
                    nc.tensor.matmul(
                        q2n[:],
                        h3[:, j : j + 1, cs],
                        wsm_sb[:, 20 + j : 21 + j],
                        start=(j == 0),
                        stop=(j == 1),
                    )
                lg2 = recp.tile([128, 1], dt.float32, tag="lg2")
                nc.vector.tensor_add(lg2[:], lgq1[:, cc : cc + 1], q2n[:])
                nc.scalar.activation(
                    out_sb[:, c : c + 1], lg2[:], AF.Sigmoid, bias=float(sig_bias)
                )

        nc.sync.dma_start(out_d[:], out_sb[:])

    nc.compile()
    # queue_num must match Tile's round-robin DMASW sem-lane assignment
    # (visit order != emission order), or one sem lane would see increments
    # from two SWDGE queues.
    for blk in nc.main_func.blocks:
        for inst in blk.instructions:
            if type(inst).__name__ == "InstDMAGatherAnt":
                si = inst.sync_info
                assert si is not None and si.on_update, inst.name
                name = si.on_update[0].ant_name
                assert name.startswith("DMASW"), name
                inst.queue_num = int(name[5:].split("_")[0]) % 4
    return nc


def _prep_inputs(
    dense_x,
    one_hot_x,
    multi_hot_x,
    one_hot_emb,
    multi_hot_emb,
    cross_w,
    cross_b,
    W1,
    b1,
    W2,
    b2,
    W3,
    b3,
    lin_w,
    lin_b,
):
    dense_bf = np.ascontiguousarray(dense_x, dtype=np.float32).astype(BF16)
    # fp32 2-row super-row tables with a wraparound copy of the low half
    tab = np.concatenate(
        [
            np.asarray(one_hot_emb, np.float32).reshape(N_OH, VOCAB, EMB),
            np.asarray(multi_hot_emb, np.float32).reshape(N_MH, VOCAB, EMB),
        ]
    ).reshape(N_OH + N_MH, SROWS, 64)
    tab2 = np.ascontiguousarray(
        np.concatenate([tab, tab[:, :SHALF, :]], axis=1), dtype=np.float32
    )

    oh_idx = np.asarray(one_hot_x, np.int64)  # (B, 8)
    mh_idx = np.asarray(multi_hot_x, np.int64)  # (B, 4, 20)

    def pack_k(Wmat, out_cols):
        p = np.zeros((128, 4, out_cols), np.float32)
        for j in range(4):
            p[0 : KS[j], j, :] = Wmat[j * 128 : j * 128 + KS[j], :]
        return p.astype(BF16)

    w1p = pack_k(np.asarray(W1, np.float32), 1024)
    w2p = (
        np.asarray(W2, np.float32)
        .reshape(8, 128, 512)
        .transpose(1, 0, 2)
        .copy()
        .astype(BF16)
    )
    w3p = (
        np.asarray(W3, np.float32)
        .reshape(4, 128, 256)
        .transpose(1, 0, 2)
        .copy()
        .astype(BF16)
    )
    lw = np.asarray(lin_w, np.float32)[:, 0]
    cwq = pack_k(
        np.concatenate([np.asarray(cross_w, np.float32).T, lw[:IN_DIM, None]], 1), 5
    )
    wsm = np.zeros((128, 22), np.float32)
    wsm[:, 0:20] = cwq.astype(np.float32).reshape(128, 20)
    wsm[:, 20:22] = lw[IN_DIM:].reshape(2, 128).T
    wsm = wsm.astype(BF16)
    biasp = np.concatenate(
        [
            np.asarray(b1, np.float32).reshape(8, 128).T,
            np.asarray(b2, np.float32).reshape(4, 128).T,
            np.asarray(b3, np.float32).reshape(2, 128).T,
        ],
        axis=1,
    ).copy()

    cb = np.asarray(cross_b, np.float64)
    cwf = np.asarray(cross_w, np.float64)
    C = np.zeros(IN_DIM, np.float64)
    c_consts = []
    for l in range(4):
        c_consts.append(float(C @ cwf[l]))
        C = C + cb[l]
    sig_bias = float(C @ np.asarray(lw[:IN_DIM], np.float64)) + float(
        np.asarray(lin_b, np.float64).reshape(-1)[0]
    )
    if any(abs(c) > 1e-30 for c in c_consts):
        raise NotImplementedError("cross_b != 0 unsupported (always 0 here)")

    shared = {
        "tab2": tab2,
        "w1p": w1p,
        "w2p": w2p,
        "w3p": w3p,
        "wsm": wsm,
        "biasp": biasp,
    }
    pad = np.zeros(16, np.int16)
    in_maps = []
    for core in range(N_CORES):
        rs = slice(core * BL, (core + 1) * BL)
        m = dict(shared)
        m["dense"] = np.ascontiguousarray(
            dense_bf[rs].reshape(BL // 128, 128, DENSE).transpose(1, 0, 2)
        )
        # one-hot: list position t = sample_local; out[p, c] = sample c*128+p
        soh = oh_idx[rs]  # (BL, 8)
        ohidx = np.stack(
            [
                _wrap_idx(
                    np.concatenate([((soh[:, f] >> 1) - SHALF).astype(np.int16), pad])
                )
                for f in range(N_OH)
            ],
            axis=1,
        )  # [128, 8, 129]
        m["ohidx"] = np.ascontiguousarray(ohidx)
        sub_oh = (soh & 1).reshape(BL // 128, 128, N_OH).transpose(1, 0, 2)
        m["ohflg"] = np.ascontiguousarray(
            (sub_oh[:, :, :, None] == np.arange(2)).astype(np.float32)
        )  # [128, 16, 8, 2]
        # multi-hot: per block b, field f: t = sb*2560 + h*128 + p
        smh = mh_idx[rs]  # (BL, 4, 20)
        mhidx = np.zeros((128, N_GB, N_MH, MH_NP // 16), np.int16)
        mhflg = np.zeros((128, N_GB, N_MH, MH_N // 128, 2), np.float32)
        for bb in range(N_GB):
            chunk = smh[bb * SBLK : (bb + 1) * SBLK]  # (256, 4, 20)
            for f in range(N_MH):
                l3 = chunk[:, f, :].reshape(2, 128, HIST).transpose(0, 2, 1)
                # l3[sb, h, p] = idx of (sample sb*128+p, hist h)
                flat = ((l3 >> 1) - SHALF).reshape(MH_N).astype(np.int16)
                mhidx[:, bb, f, :] = _wrap_idx(np.concatenate([flat, pad]))
                sub = (l3 & 1).transpose(2, 0, 1).reshape(128, MH_N // 128)
                mhflg[:, bb, f, :, :] = (sub[:, :, None] == np.arange(2)).astype(
                    np.float32
                )
        m["mhidx"] = np.ascontiguousarray(mhidx)
        m["mhflg"] = np.ascontiguousarray(mhflg)
        in_maps.append(m)
    return in_maps, c_consts, sig_bias


def _patch_interp_gather():
    """Dev-only CoreSim patch: the stock interp asserts gather indices are
    >= -1; our signed indices are valid on HW (signed Q7 address math) and
    the wraparound table copy makes numpy negative indexing agree."""
    import einops
    from concourse import bass_interp
    from concourse.bass_interp import Direction

    def cdiv(a, bb):
        return -(-a // bb)

    def patched(self, ins, captured, *, reg_snapshot):
        src_ap = self.view_ap(
            ins.ins[:-2], Direction.READ, ins, reg_snapshot=reg_snapshot
        )
        idxs_ap, _ = captured
        dst_ap = self.view_ap(
            ins.outs[0], Direction.WRITE, ins, reg_snapshot=reg_snapshot
        )
        assert not ins.transpose
        src = src_ap.reshape((-1, ins.elem_size))
        idxs = idxs_ap.reshape((128, cdiv(ins.num_idxs, 16)))
        dst = dst_ap.reshape((128, cdiv(ins.num_idxs, 128), ins.elem_size))
        unwrapped = einops.rearrange(idxs[:16, :], "p s -> (s p)")[: ins.num_idxs]
        gathered = src[unwrapped]
        n = ins.num_idxs
        full = n // 128 * 128
        if full:
            dst[:, : n // 128, :] = (
                gathered[:full]
                .reshape(n // 128, 128, ins.elem_size)
                .transpose(1, 0, 2)
            )
        for i in range(full, n):
            dst[i % 128, i // 128, :] = gathered[i]

    bass_interp.InstructionExecutor._exec_InstDMAGatherAnt = patched


def _run(inputs, trace=False, sim=False):
    from concourse.bass_utils import run_bass_kernel_spmd

    in_maps, c_consts, sig_bias = _prep_inputs(**inputs)
    nc = _build_program(c_consts, sig_bias)
    if sim:
        _patch_interp_gather()
        from concourse.bass_interp import CoreSim

        csim = CoreSim(nc, trace=False)
        for k, v in in_maps[0].items():
            csim.tensor(k)[:] = v
        csim.simulate()
        out0 = np.asarray(csim.tensor("out"))
        outs = [out0.reshape(128, BL // 128).T.reshape(BL)]
        full = np.concatenate(outs).reshape(-1, 1).astype(np.float32)
        return full, None
    res = run_bass_kernel_spmd(nc, in_maps, core_ids=list(range(N_CORES)), trace=trace)
    outs = [
        res.results[c]["out"].reshape(128, BL // 128).T.reshape(BL)
        for c in range(N_CORES)
    ]
    full = np.concatenate(outs).reshape(B, 1).astype(np.float32)
    return full, res


def kernel(**inputs):
    full, _ = _run(inputs, trace=False)
    return full


# revision 5
# speedup vs baseline: 1.1298x; 1.0082x over previous
"""DCN-v1 (dense_mlp) Trainium2 kernel.

Measured: ~0.57-0.62 ms HW exec (8 cores), rel err (l2) ~1.15e-3 — ~3.4x
faster than the indirect-DMA version (2.04 ms), whose per-128-row
instruction cost was the floor.

Strategy (8 NeuronCores, SPMD, data-parallel over batch; 2048 rows/core):
  - Embedding lookups via InstDMAGatherAnt (mlp Q7 library): one instruction
    moves thousands of table rows (vs 128/instruction for indirect DMA whose
    ~1.45us fixed cost bound the previous kernel at ~2.0ms).
    single_packet=False — coalescing >64 descriptors into one SDMA packet
    wedges the exec unit (NRT_EXEC_UNIT_UNRECOVERABLE).
  - 4 SWDGE queues (queue_num spread matching Tile's 8 round-robin DMASW
    lanes): SDMA engines round-robin the 4 rings, overlapping the ~125ns
    per-descriptor read round-trips that serialize one queue
    (measured 7.9 -> 2.6 ns/lookup).
  - Tables stored fp32 as 2-row 256B "super-rows" (the dma_gather element
    minimum) with SIGNED int16 super-indices: sidx = (idx>>1) - 25000 in
    [-25000, 25000); the Q7 address math (IVP_MULUSAN) is signed, so the
    src AP is based at row 25000. Each gather list is padded with 16
    trailing sidx=0 entries so the ucode's trailing-negative trim can
    never drop real lookups.
  - Sub-row select (idx&1) on DVE: one broadcast-flag multiply + one
    contiguous fold add; multi-hot histories then sum via a pairwise add
    tree (contiguous 32-elem inner dim — strided tensor_reduce ran ~7x
    below the contiguous DVE rate).
  - x0 assembly + PSUM eviction on the Scalar engine (ACT); feature-major
    x0^T via PE transposes; collapsed CrossNet (one 448->5 projection +
    scalar recurrence); bf16 MLP with fp32 PSUM accumulation.
"""

import os
import sys

import numpy as np
import ml_dtypes

for _p in ("/opt/trn_rl_repo", os.path.expanduser("~/.axon_site/_ro/trn_rl_repo")):
    if os.path.isdir(_p) and _p not in sys.path:
        sys.path.append(_p)

B = 16384
N_CORES = 8
BL = B // N_CORES  # 2048 rows per core
DENSE = 64
N_OH, N_MH, HIST = 8, 4, 20
VOCAB = 100000
SROWS = VOCAB // 2  # 50000 2-row super-rows per field
SHALF = SROWS // 2  # 25000: signed-index base row
EMB = 32
IN_DIM = 448
HID = [1024, 512, 256]
CHUNK = 128  # samples per transpose chunk
NBLK = 512  # samples per matmul n-block
SBLK = 256  # samples per mh gather block
N_GB = BL // SBLK  # 8 gather blocks per core
MH_N = SBLK * HIST  # 5120 real idxs per mh gather
MH_NP = MH_N + 16  # padded with trailing sidx=0
OH_N = BL  # 2048 real idxs per oh gather
OH_NP = OH_N + 16
KS = [128, 128, 128, 64]  # k-tile sizes over the 448-dim input features
BF16 = ml_dtypes.bfloat16


def _wrap_idx(flat):
    """[n] int16 -> [128, n//16] wrapped: position t at [t%16, t//16],
    replicated across the 8 16-partition groups."""
    n = flat.shape[0]
    w = flat.reshape(n // 16, 16).T  # [16, n//16]
    return np.tile(w, (8, 1)).astype(np.int16)


def _build_program(c_consts, sig_bias):
    from contextlib import ExitStack

    import concourse.bass as bass
    import concourse.tile as tile
    from concourse import bacc, mybir
    from concourse.masks import make_identity

    dt = mybir.dt
    AF = mybir.ActivationFunctionType
    n_chunks_per_nb = NBLK // CHUNK  # 4
    n_nb = BL // NBLK  # 4

    nc = bacc.Bacc(dynamic_dma_scratch_size=49152, num_swdge_queues=4)
    # per field: [50000 super-rows | copy of the first 25000] — the copy makes
    # numpy-style negative indexing in CoreSim land on the same rows the HW's
    # signed offset from base row 25000 reaches.
    tab_d = nc.dram_tensor(
        "tab2", [N_OH + N_MH, SROWS + SHALF, 64], dt.float32, kind="ExternalInput"
    )
    dense_d = nc.dram_tensor(
        "dense", [128, BL // 128, DENSE], dt.bfloat16, kind="ExternalInput"
    )
    ohi_d = nc.dram_tensor(
        "ohidx", [128, N_OH, OH_NP // 16], dt.int16, kind="ExternalInput"
    )
    mhi_d = nc.dram_tensor(
        "mhidx", [128, N_GB, N_MH, MH_NP // 16], dt.int16, kind="ExternalInput"
    )
    ohf_d = nc.dram_tensor(
        "ohflg", [128, BL // 128, N_OH, 2], dt.float32, kind="ExternalInput"
    )
    mhf_d = nc.dram_tensor(
        "mhflg", [128, N_GB, N_MH, MH_N // 128, 2], dt.float32, kind="ExternalInput"
    )
    w1_d = nc.dram_tensor("w1p", [128, 4, 1024], dt.bfloat16, kind="ExternalInput")
    w2_d = nc.dram_tensor("w2p", [128, 8, 512], dt.bfloat16, kind="ExternalInput")
    w3_d = nc.dram_tensor("w3p", [128, 4, 256], dt.bfloat16, kind="ExternalInput")
    wsm_d = nc.dram_tensor("wsm", [128, 22], dt.bfloat16, kind="ExternalInput")
    bias_d = nc.dram_tensor("biasp", [128, 14], dt.float32, kind="ExternalInput")
    out_d = nc.dram_tensor("out", [128, BL // 128], dt.float32, kind="ExternalOutput")

    with ExitStack() as ctx:
        tc = ctx.enter_context(tile.TileContext(nc))
        wp = ctx.enter_context(tc.tile_pool(name="weights", bufs=1))
        ohrp = ctx.enter_context(tc.tile_pool(name="ohr", bufs=2))
        mhrp = ctx.enter_context(tc.tile_pool(name="mhr", bufs=6))
        prodp = ctx.enter_context(tc.tile_pool(name="prod", bufs=2))
        r1p = ctx.enter_context(tc.tile_pool(name="r1", bufs=2))
        poolp = ctx.enter_context(tc.tile_pool(name="pool", bufs=2))
        x0p = ctx.enter_context(tc.tile_pool(name="x0", bufs=2))
        xtp = ctx.enter_context(tc.tile_pool(name="xt", bufs=2))
        hp = ctx.enter_context(tc.tile_pool(name="h", bufs=1))
        recp = ctx.enter_context(tc.tile_pool(name="rec", bufs=2))
        ps_mm = ctx.enter_context(tc.tile_pool(name="psmm", bufs=3, space="PSUM"))
        ps_tr = ctx.enter_context(tc.tile_pool(name="pstr", bufs=2, space="PSUM"))
        ps_sm = ctx.enter_context(tc.tile_pool(name="pssm", bufs=1, space="PSUM"))
        ps_q2 = ctx.enter_context(tc.tile_pool(name="psq2", bufs=2, space="PSUM"))

        # --- resident weights / indices / flags ---
        w1_sb = wp.tile([128, 4, 1024], dt.bfloat16)
        nc.sync.dma_start(w1_sb[:], w1_d[:])
        w2_sb = wp.tile([128, 8, 512], dt.bfloat16)
        nc.sync.dma_start(w2_sb[:], w2_d[:])
        w3_sb = wp.tile([128, 4, 256], dt.bfloat16)
        nc.sync.dma_start(w3_sb[:], w3_d[:])
        wsm_sb = wp.tile([128, 22], dt.bfloat16)
        nc.sync.dma_start(wsm_sb[:], wsm_d[:])
        bias_sb = wp.tile([128, 14], dt.float32)
        nc.sync.dma_start(bias_sb[:], bias_d[:])
        ident = wp.tile([128, 128], dt.bfloat16)
        make_identity(nc, ident[:])
        dense_sb = wp.tile([128, BL // 128, DENSE], dt.bfloat16)
        nc.sync.dma_start(dense_sb[:], dense_d[:])
        ohi_sb = wp.tile([128, N_OH, OH_NP // 16], dt.int16)
        nc.sync.dma_start(ohi_sb[:], ohi_d[:])
        ohf_sb = wp.tile([128, BL // 128, N_OH, 2], dt.float32)
        nc.sync.dma_start(ohf_sb[:], ohf_d[:])
        out_sb = wp.tile([128, BL // 128], dt.float32)
        # one-hot selected rows, chunk-major: [p, chunk, field, 32] bf16
        ohsel = wp.tile([128, BL // 128, N_OH, EMB], dt.bfloat16)

        src = tab_d[:, SHALF : SHALF + SROWS, :]

        # ---- one-hot: gather whole-core per field, select sub-row ----
        for f in range(N_OH):
            ohraw = ohrp.tile([128, OH_NP // 128 + 1, 64], dt.float32, tag="ohraw")
            nc.gpsimd.dma_gather(
                out_ap=ohraw[:],
                in_ap=src[f, :, :],
                idxs_ap=ohi_sb[:, f, :],
                num_idxs=OH_NP,
                num_idxs_reg=OH_NP,
                elem_size=64,
                elem_step=64,
                transpose=False,
                single_packet=False,
                queue_num=f % 4,
            )
            ohprod = ohrp.tile([128, BL // 128, 64], dt.float32, tag="ohprod")
            nc.vector.tensor_mul(
                ohprod[:].rearrange("p c (s e) -> p c s e", s=2),
                ohraw[:, 0 : BL // 128, :].rearrange("p c (s e) -> p c s e", s=2),
                ohf_sb[:, :, f, :].broadcast_to([128, BL // 128, 2, EMB]),
            )
            nc.vector.tensor_add(
                ohsel[:, :, f, :], ohprod[:, :, 0:32], ohprod[:, :, 32:64]
            )

        # ---- multi-hot: per gather-block of 256 samples, per field ----
        # list position t = sb*2560 + h*128 + p -> (sample sb*128+p, hist h);
        # gather layout puts it at [p, sb*20+h], so each sample's 20 rows are
        # contiguous in one partition's free dim.
        pooled = {}
        for b in range(N_GB):
            pb = poolp.tile([128, 2, N_MH, EMB], dt.float32, tag="pooled")
            pooled[b] = pb
            mhf_sb = r1p.tile([128, N_MH, MH_N // 128, 2], dt.float32, tag="mhf")
            nc.sync.dma_start(mhf_sb[:], mhf_d[:, b])
            mhi_sb = r1p.tile([128, N_MH, MH_NP // 16], dt.int16, tag="mhi")
            nc.sync.dma_start(mhi_sb[:], mhi_d[:, b])
            for f in range(N_MH):
                mhraw = mhrp.tile(
                    [128, MH_NP // 128 + 1, 64], dt.float32, tag="mhraw"
                )
                nc.gpsimd.dma_gather(
                    out_ap=mhraw[:],
                    in_ap=src[N_OH + f, :, :],
                    idxs_ap=mhi_sb[:, f, :],
                    num_idxs=MH_NP,
                    num_idxs_reg=MH_NP,
                    elem_size=64,
                    elem_step=64,
                    transpose=False,
                    single_packet=False,
                    queue_num=(b * N_MH + f) % 4,
                )
                prod = prodp.tile([128, MH_N // 128, 64], dt.float32, tag="mhprod")
                nc.vector.tensor_mul(
                    prod[:].rearrange("p j (s e) -> p j s e", s=2),
                    mhraw[:, 0 : MH_N // 128, :].rearrange(
                        "p j (s e) -> p j s e", s=2
                    ),
                    mhf_sb[:, f, :, :].broadcast_to([128, MH_N // 128, 2, EMB]),
                )
                # fold the 2 sub-rows, then a pairwise tree over hist — every
                # add has a contiguous 32-elem inner dim.
                s2 = r1p.tile([128, MH_N // 128, 32], dt.bfloat16, tag="s2")
                nc.vector.tensor_add(s2[:], prod[:, :, 0:32], prod[:, :, 32:64])
                s2v = s2[:].rearrange("p (sb h) e -> p sb h e", sb=2)
                t1 = r1p.tile([128, 2, 10, 32], dt.bfloat16, tag="t1")
                nc.vector.tensor_add(t1[:], s2v[:, :, 0::2, :], s2v[:, :, 1::2, :])
                t2 = r1p.tile([128, 2, 5, 32], dt.bfloat16, tag="t2")
                nc.vector.tensor_add(t2[:], t1[:, :, 0::2, :], t1[:, :, 1::2, :])
                t3 = r1p.tile([128, 2, 2, 32], dt.bfloat16, tag="t3")
                nc.vector.tensor_add(t3[:], t2[:, :, 0:4:2, :], t2[:, :, 1:4:2, :])
                t4 = r1p.tile([128, 2, 32], dt.float32, tag="t4")
                nc.vector.tensor_add(t4[:], t3[:, :, 0, :], t3[:, :, 1, :])
                nc.vector.tensor_add(pb[:, :, f, :], t4[:], t2[:, :, 4, :])

        # ---- per n-block: build x0^T, cross projections, MLP, output ----
        for nb in range(n_nb):
            x0T = xtp.tile([128, 4, NBLK], dt.bfloat16, tag="x0T")
            lgq1 = recp.tile([128, 4], dt.float32, tag="lgq1")
            for cc in range(n_chunks_per_nb):
                c = nb * n_chunks_per_nb + cc
                cs = slice(cc * CHUNK, (cc + 1) * CHUNK)
                b, sb = c // 2, c % 2

                x0c = x0p.tile([128, 512], dt.bfloat16, tag="x0c")
                nc.vector.memset(x0c[:, 448:512], 0.0)
                nc.scalar.activation(
                    x0c[:, 0:DENSE], dense_sb[:, c, :], AF.Copy, bias=0.0
                )
                nc.scalar.activation(
                    x0c[:, DENSE : DENSE + N_OH * EMB].rearrange(
                        "p (f e) -> p f e", f=N_OH
                    ),
                    ohsel[:, c, :, :],
                    AF.Copy,
                    bias=0.0,
                )
                nc.scalar.activation(
                    x0c[:, 320:448].rearrange("p (f e) -> p f e", f=N_MH),
                    pooled[b][:, sb, :, :],
                    AF.Copy,
                    bias=0.0,
                )

                tp = ps_tr.tile([128, 4, 128], dt.bfloat16, tag="trps")
                for j in range(4):
                    nc.tensor.transpose(
                        tp[:, j : j + 1, :],
                        x0c[:, j * 128 : (j + 1) * 128],
                        ident[:],
                    )
                nc.scalar.activation(x0T[:, :, cs], tp[:], AF.Copy, bias=0.0)

                pn = ps_sm.tile([128, 5], dt.float32, tag="pn")
                for j in range(4):
                    nc.tensor.matmul(
                        pn[:],
                        x0T[0 : KS[j], j : j + 1, cs],
                        wsm_sb[0 : KS[j], j * 5 : j * 5 + 5],
                        start=(j == 0),
                        stop=(j == 3),
                    )
                pp1 = recp.tile([128, 4], dt.float32, tag="pp1")
                nc.vector.tensor_scalar_add(pp1[:], pn[:, 0:4], 1.0)
                m01 = recp.tile([128, 1], dt.float32, tag="m01")
                nc.vector.tensor_mul(m01[:], pp1[:, 0:1], pp1[:, 1:2])
                m23 = recp.tile([128, 1], dt.float32, tag="m23")
                nc.vector.tensor_mul(m23[:], pp1[:, 2:3], pp1[:, 3:4])
                a4 = recp.tile([128, 1], dt.float32, tag="a4")
                nc.vector.tensor_mul(a4[:], m01[:], m23[:])
                nc.vector.tensor_mul(lgq1[:, cc : cc + 1], a4[:], pn[:, 4:5])

            # ---- deep net ----
            h1 = hp.tile([128, 8, NBLK], dt.bfloat16, tag="h1")
            for m in range(8):
                ps = ps_mm.tile([128, NBLK], dt.float32, tag="mm")
                for j in range(4):
                    nc.tensor.matmul(
                        ps[:],
                        w1_sb[0 : KS[j], j : j + 1, m * 128 : (m + 1) * 128],
                        x0T[0 : KS[j], j : j + 1, :],
                        start=(j == 0),
                        stop=(j == 3),
                    )
                nc.scalar.activation(
                    h1[:, m : m + 1, :], ps[:], AF.Relu, bias=bias_sb[:, m : m + 1]
                )
            h2 = hp.tile([128, 4, NBLK], dt.bfloat16, tag="h2")
            for m in range(4):
                ps = ps_mm.tile([128, NBLK], dt.float32, tag="mm")
                for j in range(8):
                    nc.tensor.matmul(
                        ps[:],
                        w2_sb[:, j : j + 1, m * 128 : (m + 1) * 128],
                        h1[:, j : j + 1, :],
                        start=(j == 0),
                        stop=(j == 7),
                    )
                nc.scalar.activation(
                    h2[:, m : m + 1, :], ps[:], AF.Relu, bias=bias_sb[:, 8 + m : 9 + m]
                )
            h3 = hp.tile([128, 2, NBLK], dt.bfloat16, tag="h3")
            for m in range(2):
                ps = ps_mm.tile([128, NBLK], dt.float32, tag="mm")
                for j in range(4):
                    nc.tensor.matmul(
                        ps[:],
                        w3_sb[:, j : j + 1, m * 128 : (m + 1) * 128],
                        h2[:, j : j + 1, :],
                        start=(j == 0),
                        stop=(j == 3),
                    )
                nc.scalar.activation(
                    h3[:, m : m + 1, :],
                    ps[:],
                    AF.Relu,
                    bias=bias_sb[:, 12 + m : 13 + m],
                )

            # ---- final: logit = prod(1+p)*q1 + h3.lin_w_bot + sig_bias ----
            for cc in range(n_chunks_per_nb):
                c = nb * n_chunks_per_nb + cc
                cs = slice(cc * CHUNK, (cc + 1) * CHUNK)
                q2n = ps_q2.tile([128, 1], dt.float32, tag="q2n")
                for j in range(2):
                    nc.tensor.matmul(
                        q2n[:],
                        h3[:, j : j + 1, cs],
                        wsm_sb[:, 20 + j : 21 + j],
                        start=(j == 0),
                        stop=(j == 1),
                    )
                lg2 = recp.tile([128, 1], dt.float32, tag="lg2")
                nc.vector.tensor_add(lg2[:], lgq1[:, cc : cc + 1], q2n[:])
                nc.scalar.activation(
                    out_sb[:, c : c + 1], lg2[:], AF.Sigmoid, bias=float(sig_bias)
                )

        nc.sync.dma_start(out_d[:], out_sb[:])

    nc.compile()
    # queue_num must match Tile's round-robin DMASW sem-lane assignment
    # (visit order != emission order), or one sem lane would see increments
    # from two SWDGE queues.
    for blk in nc.main_func.blocks:
        for inst in blk.instructions:
            if type(inst).__name__ == "InstDMAGatherAnt":
                si = inst.sync_info
                assert si is not None and si.on_update, inst.name
                name = si.on_update[0].ant_name
                assert name.startswith("DMASW"), name
                inst.queue_num = int(name[5:].split("_")[0]) % 4
    return nc


def _prep_inputs(
    dense_x,
    one_hot_x,
    multi_hot_x,
    one_hot_emb,
    multi_hot_emb,
    cross_w,
    cross_b,
    W1,
    b1,
    W2,
    b2,
    W3,
    b3,
    lin_w,
    lin_b,
):
    dense_bf = np.ascontiguousarray(dense_x, dtype=np.float32).astype(BF16)
    # fp32 2-row super-row tables with a wraparound copy of the low half
    tab = np.concatenate(
        [
            np.asarray(one_hot_emb, np.float32).reshape(N_OH, VOCAB, EMB),
            np.asarray(multi_hot_emb, np.float32).reshape(N_MH, VOCAB, EMB),
        ]
    ).reshape(N_OH + N_MH, SROWS, 64)
    tab2 = np.ascontiguousarray(
        np.concatenate([tab, tab[:, :SHALF, :]], axis=1), dtype=np.float32
    )

    oh_idx = np.asarray(one_hot_x, np.int64)  # (B, 8)
    mh_idx = np.asarray(multi_hot_x, np.int64)  # (B, 4, 20)

    def pack_k(Wmat, out_cols):
        p = np.zeros((128, 4, out_cols), np.float32)
        for j in range(4):
            p[0 : KS[j], j, :] = Wmat[j * 128 : j * 128 + KS[j], :]
        return p.astype(BF16)

    w1p = pack_k(np.asarray(W1, np.float32), 1024)
    w2p = (
        np.asarray(W2, np.float32)
        .reshape(8, 128, 512)
        .transpose(1, 0, 2)
        .copy()
        .astype(BF16)
    )
    w3p = (
        np.asarray(W3, np.float32)
        .reshape(4, 128, 256)
        .transpose(1, 0, 2)
        .copy()
        .astype(BF16)
    )
    lw = np.asarray(lin_w, np.float32)[:, 0]
    cwq = pack_k(
        np.concatenate([np.asarray(cross_w, np.float32).T, lw[:IN_DIM, None]], 1), 5
    )
    wsm = np.zeros((128, 22), np.float32)
    wsm[:, 0:20] = cwq.astype(np.float32).reshape(128, 20)
    wsm[:, 20:22] = lw[IN_DIM:].reshape(2, 128).T
    wsm = wsm.astype(BF16)
    biasp = np.concatenate(
        [
            np.asarray(b1, np.float32).reshape(8, 128).T,
            np.asarray(b2, np.float32).reshape(4, 128).T,
            np.asarray(b3, np.float32).reshape(2, 128).T,
        ],
        axis=1,
    ).copy()

    cb = np.asarray(cross_b, np.float64)
    cwf = np.asarray(cross_w, np.float64)
    C = np.zeros(IN_DIM, np.float64)
    c_consts = []
    for l in range(4):
        c_consts.append(float(C @ cwf[l]))
        C = C + cb[l]
    sig_bias = float(C @ np.asarray(lw[:IN_DIM], np.float64)) + float(
        np.asarray(lin_b, np.float64).reshape(-1)[0]
    )
    if any(abs(c) > 1e-30 for c in c_consts):
        raise NotImplementedError("cross_b != 0 unsupported (always 0 here)")

    shared = {
        "tab2": tab2,
        "w1p": w1p,
        "w2p": w2p,
        "w3p": w3p,
        "wsm": wsm,
        "biasp": biasp,
    }
    pad = np.zeros(16, np.int16)
    in_maps = []
    for core in range(N_CORES):
        rs = slice(core * BL, (core + 1) * BL)
        m = dict(shared)
        m["dense"] = np.ascontiguousarray(
            dense_bf[rs].reshape(BL // 128, 128, DENSE).transpose(1, 0, 2)
        )
        # one-hot: list position t = sample_local; out[p, c] = sample c*128+p
        soh = oh_idx[rs]  # (BL, 8)
        ohidx = np.stack(
            [
                _wrap_idx(
                    np.concatenate([((soh[:, f] >> 1) - SHALF).astype(np.int16), pad])
                )
                for f in range(N_OH)
            ],
            axis=1,
        )  # [128, 8, 129]
        m["ohidx"] = np.ascontiguousarray(ohidx)
        sub_oh = (soh & 1).reshape(BL // 128, 128, N_OH).transpose(1, 0, 2)
        m["ohflg"] = np.ascontiguousarray(
            (sub_oh[:, :, :, None] == np.arange(2)).astype(np.float32)
        )  # [128, 16, 8, 2]
        # multi-hot: per block b, field f: t = sb*2560 + h*128 + p
        smh = mh_idx[rs]  # (BL, 4, 20)
        mhidx = np.zeros((128, N_GB, N_MH, MH_NP // 16), np.int16)
        mhflg = np.zeros((128, N_GB, N_MH, MH_N // 128, 2), np.float32)
        for bb in range(N_GB):
            chunk = smh[bb * SBLK : (bb + 1) * SBLK]  # (256, 4, 20)
            for f in range(N_MH):
                l3 = chunk[:, f, :].reshape(2, 128, HIST).transpose(0, 2, 1)
                # l3[sb, h, p] = idx of (sample sb*128+p, hist h)
                flat = ((l3 >> 1) - SHALF).reshape(MH_N).astype(np.int16)
                mhidx[:, bb, f, :] = _wrap_idx(np.concatenate([flat, pad]))
                sub = (l3 & 1).transpose(2, 0, 1).reshape(128, MH_N // 128)
                mhflg[:, bb, f, :, :] = (sub[:, :, None] == np.arange(2)).astype(
                    np.float32
                )
        m["mhidx"] = np.ascontiguousarray(mhidx)
        m["mhflg"] = np.ascontiguousarray(mhflg)
        in_maps.append(m)
    return in_maps, c_consts, sig_bias


def _patch_interp_gather():
    """Dev-only CoreSim patch: the stock interp asserts gather indices are
    >= -1; our signed indices are valid on HW (signed Q7 address math) and
    the wraparound table copy makes numpy negative indexing agree."""
    import einops
    from concourse import bass_interp
    from concourse.bass_interp import Direction

    def cdiv(a, bb):
        return -(-a // bb)

    def patched(self, ins, captured, *, reg_snapshot):
        src_ap = self.view_ap(
            ins.ins[:-2], Direction.READ, ins, reg_snapshot=reg_snapshot
        )
        idxs_ap, _ = captured
        dst_ap = self.view_ap(
            ins.outs[0], Direction.WRITE, ins, reg_snapshot=reg_snapshot
        )
        assert not ins.transpose
        src = src_ap.reshape((-1, ins.elem_size))
        idxs = idxs_ap.reshape((128, cdiv(ins.num_idxs, 16)))
        dst = dst_ap.reshape((128, cdiv(ins.num_idxs, 128), ins.elem_size))
        unwrapped = einops.rearrange(idxs[:16, :], "p s -> (s p)")[: ins.num_idxs]
        gathered = src[unwrapped]
        n = ins.num_idxs
        full = n // 128 * 128
        if full:
            dst[:, : n // 128, :] = (
                gathered[:full]
                .reshape(n // 128, 128, ins.elem_size)
                .transpose(1, 0, 2)
            )
        for i in range(full, n):
            dst[i % 128, i // 128, :] = gathered[i]

    bass_interp.InstructionExecutor._exec_InstDMAGatherAnt = patched


def _run(inputs, trace=False, sim=False):
    from concourse.bass_utils import run_bass_kernel_spmd

    in_maps, c_consts, sig_bias = _prep_inputs(**inputs)
    nc = _build_program(c_consts, sig_bias)
    if sim:
        _patch_interp_gather()
        from concourse.bass_interp import CoreSim

        csim = CoreSim(nc, trace=False)
        for k, v in in_maps[0].items():
            csim.tensor(k)[:] = v
        csim.simulate()
        out0 = np.asarray(csim.tensor("out"))
        outs = [out0.reshape(128, BL // 128).T.reshape(BL)]
        full = np.concatenate(outs).reshape(-1, 1).astype(np.float32)
        return full, None
    res = run_bass_kernel_spmd(nc, in_maps, core_ids=list(range(N_CORES)), trace=trace)
    outs = [
        res.results[c]["out"].reshape(128, BL // 128).T.reshape(BL)
        for c in range(N_CORES)
    ]
    full = np.concatenate(outs).reshape(B, 1).astype(np.float32)
    return full, res


def kernel(**inputs):
    full, _ = _run(inputs, trace=False)
    return full
